# revision 13
# baseline (speedup 1.0000x reference)
"""v4: bf16 x + fp8 xT feed; r-pass moved onto the PE via DoubleRow.

Same math as kernel.py (z = X^T diag(f) X trick, zero collectives).
Differences vs v2.5:
 - host also feeds xT (x transposed, e4m3 fp8, column order k = p*64+i so
   the r-vector redistributes to row layout with 16 contiguous-run DMA
   descriptors per chunk).
 - r = X.t is computed on the PE: lhsT = tT pair (d-halves interleaved,
   DoubleRow), rhs = xT [128,2,1024] chunks -> r broadcast in PSUM.
   t is split hi+lo e4m3 (lo = residual, accumulated into the same PSUM)
   because a single e4m3 quantization of t costs 1e-2 rel err.
 - f computed in one batch after a small distribute; g/z pipeline follows.
"""

import os
import sys

import numpy as np

for _p in ("/opt/trn_rl_repo", "/root/.axon_site/_ro/trn_rl_repo"):
    if os.path.isdir(_p) and _p not in sys.path:
        sys.path.insert(0, _p)

import concourse.bacc as bacc
import concourse.mybir as mybir
import concourse.tile as tile
import concourse.masks as masks
from concourse import bass_utils
from concourse.bass_types import AP as _AP

# NOTE: --enable-ldw-opt=true crashes walrus (visitInstLdweights); the
# exposed LDWEIGHTS cost must be hidden by scheduling instead.

import ml_dtypes

R = 8
N, D = 8192, 256
NL = N // R
P = 128
T = N // P             # 64 row tiles
TL = NL // P           # 8 local row tiles
CH = 8                 # tiles per chunk
NCH = T // CH          # 8 chunks
CW = CH * D
KC = 512               # xT k-chunk width (16 chunks; 1 PSUM bank each)
NKC = N // KC
F32 = mybir.dt.float32
BF16 = mybir.dt.bfloat16
FP8 = mybir.dt.float8e4
AF = mybir.ActivationFunctionType
ALU = mybir.AluOpType
DR = mybir.MatmulPerfMode.DoubleRow

_cache = {}


def _program(tc, x, xT, W, out):
    nc = tc.nc
    with (
        tc.tile_pool(name="persist", bufs=1) as pp,
        tc.tile_pool(name="work", bufs=4) as wp,
        tc.tile_pool(name="psA", bufs=1, space="PSUM") as psA,
        tc.tile_pool(name="psR", bufs=4, space="PSUM") as psR,
        tc.tile_pool(name="psW", bufs=2, space="PSUM") as psW,
    ):
        xb_all = pp.tile([P, T * D], BF16)       # x row-layout (4MB)
        xT_all = pp.tile([P, 2 * N], FP8)        # xT: half h at col h*N (2MB)
        nsq = pp.tile([P, T], F32)
        nrm = pp.tile([P, T], F32)
        invn_bf = pp.tile([P, T], BF16)
        r_dist = pp.tile([P, T], F32)            # r in row layout
        p_t = pp.tile([P, T], F32)
        sp_t = pp.tile([P, T], F32)
        f_t = pp.tile([P, T], F32)

        Wb_sb = pp.tile([P, 2 * D], BF16)
        t_sb = pp.tile([P, D], F32)              # t broadcast (f32)
        tb_sb = pp.tile([P, D], BF16)
        tT8 = pp.tile([P, 64], FP8)              # hi: cols 0,16; lo: cols 32,48
        ident_f = pp.tile([P, P], F32)
        ident_bf = pp.tile([P, P], BF16)
        xbT = pp.tile([P, 2 * TL * P], BF16)
        z_top_sb = pp.tile([P, D], BF16)
        z22_sb = pp.tile([P, P], BF16)
        z21_sb = pp.tile([P, P], BF16)
        zw_sb = pp.tile([P, 2 * D], BF16)

        # tb banks live in the psR rotation ring: r-chunk 2 reuses tb_ps0's
        # bank only after the t-combine read, which the rotation enforces
        tb_ps0 = psR.tile([P, D], F32, tag="r", name="tb_ps0")
        tb_ps1 = psR.tile([P, D], F32, tag="r", name="tb_ps1")
        z_top_ps = psA.tile([P, D], F32, name="z_top_ps")
        z22_ps = psA.tile([P, P], F32, name="z22_ps")

        # ---- input DMAs ----
        for c in range(NCH):
            src = x[c * CH * P:(c + 1) * CH * P, :].rearrange(
                "(p j) d -> p j d", p=P
            )
            nc.sync.dma_start(
                xb_all[:, c * CW:(c + 1) * CW].rearrange("p (j d) -> p j d", j=CH),
                src,
            )
        for kc in range(2):
            nc.sync.dma_start(Wb_sb[:, kc * D:(kc + 1) * D], W[kc * P:(kc + 1) * P, :])

        masks.make_identity(nc, ident_f[:])
        nc.vector.tensor_copy(ident_bf[:], ident_f[:])
        warm = pp.tile([1, 8], F32)
        nc.scalar.activation(warm[:], ident_bf[0:1, 0:8], AF.Square)
        nc.scalar.activation(warm[:], ident_bf[0:1, 0:8], AF.Sqrt)
        for w in range(20):
            wps = psW.tile([P, P], BF16, tag="pw", name=f"warm{w}")
            nc.tensor.transpose(wps[:], ident_bf[:], ident_bf[:])

        # ---- phase A: sumsq, invn, t accumulation; xbT transposes ----
        # Software-pipelined with lag 1: the cross-engine consumers (sqrt,
        # recip, t-matmuls) of chunk c-1 are issued AFTER chunk c's sumsq
        # ops, so no engine queue ever stalls head-of-line on another
        # engine's in-flight work.
        def _a_tail(c):
            cs = slice(c * CH, (c + 1) * CH)
            nc.scalar.activation(nrm[:, cs], nsq[:, cs], AF.Sqrt)
            nc.vector.reciprocal(invn_bf[:, cs], nrm[:, cs])
            for j in range(CH):
                i = c * CH + j
                iv = invn_bf[:, i:i + 1]
                iv_rep = _AP(iv.tensor, iv.offset, [iv.ap[0], [0, P]])
                nc.tensor.matmul(
                    tb_ps0[:] if i % 2 == 0 else tb_ps1[:], lhsT=iv_rep,
                    rhs=xb_all[:, i * D:(i + 1) * D],
                    start=(i < 2), stop=(i >= T - 2),
                )

        for c in range(NCH):
            for j in range(CH):
                i = c * CH + j
                xi = xb_all[:, i * D:(i + 1) * D]
                o = nsq[:, i:i + 1]
                if j < 3:
                    ja = wp.tile([P, D], BF16, tag="ja", name=f"ja{i}")
                    nc.scalar.activation(ja[:], xi, AF.Square, accum_out=o)
                else:
                    jd = wp.tile([P, D], BF16, tag="jd", name=f"jd{i}")
                    nc.vector.scalar_tensor_tensor(
                        jd[:], xi, 0.0, xi, op0=ALU.bypass, op1=ALU.mult,
                        accum_out=o,
                    )
            if c == 5:
                # delay the xT feed until DVE reaches this point (~12us in):
                # the dummy read of xT_all forces a WAR dependency, so the x
                # chunks get the full HBM bandwidth first (xT isn't consumed
                # until the r-pass, much later)
                xjunk2 = pp.tile([1, 8], F32)
                nc.vector.tensor_copy(xjunk2[:], xT_all[0:1, 0:8])
                nc.sync.dma_start(
                    xT_all[:].rearrange("p (h k) -> p h k", h=2),
                    xT.rearrange("p h k -> p h k"),
                )
            if c >= 1:
                _a_tail(c - 1)
        _a_tail(NCH - 1)

        # ---- t fixup: combine banks, transpose to tT, build hi/lo fp8 ----
        # (walrus: only one non-scalar input may be read from PSUM)
        t_tmp = pp.tile([P, D], F32)
        nc.vector.tensor_copy(t_tmp[:], tb_ps0[:])
        nc.vector.tensor_add(t_sb[:], t_tmp[:], tb_ps1[:])
        nc.vector.tensor_copy(tb_sb[:], t_sb[:])
        for h in range(2):
            tt_ps = psW.tile([P, P], F32, tag="pw", name=f"tt{h}")
            # t_sb is partition-broadcast, so transposing any 128-col block
            # puts t[h*128+p] on partition p (all columns equal)
            nc.tensor.transpose(tt_ps[:], t_sb[:, h * P:(h + 1) * P], ident_f[:])
            # hi = fp8(t/64) at col 16*h ; lo = fp8(t/64 - hi) at col 32+16*h
            nc.vector.tensor_scalar_mul(
                tT8[:, 16 * h:16 * h + 1], tt_ps[:, 0:1], 1.0 / 64.0)
            nc.vector.scalar_tensor_tensor(
                tT8[:, 32 + 16 * h:32 + 16 * h + 1], tt_ps[:, 0:1], 1.0 / 64.0,
                tT8[:, 16 * h:16 * h + 1], op0=ALU.mult, op1=ALU.subtract,
            )

        # ---- phase B: r = X.t on PE via DoubleRow over xT k-chunks ----
        # lhsT = tT pair [128,2,128(rep)], rhs = xT pair [128,2,1024]
        # hi chain accumulates, then lo chain into the same PSUM.
        xT3 = xT_all[:].rearrange("p (h k) -> p h k", h=2)
        r_row = pp.tile([1, N], F32)   # all evac'd r, one partition
        for c in range(NKC):
            r_ps = psR.tile([P, KC], F32, tag="r", name=f"r{c}")
            for lv in range(2):  # 0=hi, 1=lo
                col = 32 * lv
                tt = tT8[:, col:col + 17]
                lhsT = _AP(tt.tensor, tt.offset, [tt.ap[0], [16, 2], [0, P]])
                rhs_slice = xT3[:, :, c * KC:(c + 1) * KC]
                nc.tensor.matmul(
                    r_ps[:], lhsT=lhsT, rhs=rhs_slice,
                    start=(lv == 0), stop=(lv == 1), perf_mode=DR,
                )
            # r is partition-broadcast; evac one partition's row to SBUF
            # (DMA cannot read PSUM); alternate engines
            if c % 2 == 0:
                nc.vector.tensor_copy(r_row[:, c * KC:(c + 1) * KC], r_ps[0:1, :])
            else:
                nc.scalar.copy(r_row[:, c * KC:(c + 1) * KC], r_ps[0:1, :])
            # xbT transposes for the final GEMM ride in this window (PE has
            # spare cycles between DR matmuls; DVE/ACT between evacs)
            i, h = c // 2, c % 2
            pt = psW.tile([P, P], BF16, tag="pw", name=f"pt{i}_{h}")
            nc.tensor.transpose(
                pt[:], xb_all[:, i * D + h * P:i * D + (h + 1) * P], ident_bf[:])
            if c % 2 == 1:
                nc.vector.tensor_copy(
                    xbT[:, (2 * i + h) * P:(2 * i + h + 1) * P], pt[:])
            else:
                nc.scalar.copy(
                    xbT[:, (2 * i + h) * P:(2 * i + h + 1) * P], pt[:])
        # keep the PE's HAM clock warm through the distribute/f gap
        for w in range(6):
            wps = psW.tile([P, P], BF16, tag="pw", name=f"gapwarm{w}")
            nc.tensor.transpose(wps[:], ident_bf[:], ident_bf[:])
        # one distribute DMA: dst[p, i] <- src[0, p*64+i]
        nc.sync.dma_start(r_dist[:], r_row[:])

        # ---- f batch: f = rsqrt(nrm * r * 64); fold 64 into the Sqrt scale
        nc.vector.tensor_mul(p_t[:], nrm[:], r_dist[:])
        nc.scalar.activation(sp_t[:], p_t[:], AF.Sqrt, scale=64.0)
        nc.vector.reciprocal(f_t[:], sp_t[:])

        # ---- phase C: g = f*x, z accumulation (lag-1 pipelined) ----
        # z matmuls for chunk c-1 are issued after chunk c's g ops, so the
        # PE queue never blocks on a just-written g tile (and LDWEIGHTS of
        # the next matmul stays hidden behind the current one).
        g_chs = {}

        def _z_mms(c):
            g_ch = g_chs[c]
            for j in range(CH):
                i = c * CH + j
                nc.tensor.matmul(
                    z_top_ps[:], lhsT=xb_all[:, i * D:i * D + P],
                    rhs=g_ch[:, j * D:(j + 1) * D],
                    start=(i == 0), stop=(i == T - 1),
                )
                nc.tensor.matmul(
                    z22_ps[:], lhsT=xb_all[:, i * D + P:(i + 1) * D],
                    rhs=g_ch[:, j * D + P:(j + 1) * D],
                    start=(i == 0), stop=(i == T - 1),
                )

        for c in range(NCH):
            g_ch = wp.tile([P, CW], BF16, tag="g", name=f"g{c}")
            g_chs[c] = g_ch
            for j in range(CH):
                i = c * CH + j
                g_i = g_ch[:, j * D:(j + 1) * D]
                if j < 2:
                    nc.scalar.mul(g_i, xb_all[:, i * D:(i + 1) * D],
                                  f_t[:, i:i + 1])
                else:
                    nc.vector.tensor_scalar_mul(g_i, xb_all[:, i * D:(i + 1) * D],
                                                f_t[:, i:i + 1])
            if c >= 1:
                _z_mms(c - 1)
        _z_mms(NCH - 1)

        # ---- zw = z @ W (symmetry: z21 = z12^T) ----
        nc.vector.tensor_copy(z_top_sb[:], z_top_ps[:])
        nc.vector.tensor_copy(z22_sb[:], z22_ps[:])
        zT_ps = psW.tile([P, P], BF16, tag="pw", name="zT")
        nc.tensor.transpose(zT_ps[:], z_top_sb[:, P:D], ident_bf[:])
        nc.vector.tensor_copy(z21_sb[:], zT_ps[:])

        for half, (lhs1, lhs2) in (
            (1, (z_top_sb[:, P:D], z22_sb[:])),
            (0, (z_top_sb[:, 0:P], z21_sb[:])),
        ):
            zw_ps = psW.tile([P, D], F32, tag="pw", name=f"zw{half}")
            nc.tensor.matmul(zw_ps[:], lhsT=lhs1, rhs=Wb_sb[:, 0:D],
                             start=True, stop=False)
            nc.tensor.matmul(zw_ps[:], lhsT=lhs2, rhs=Wb_sb[:, D:2 * D],
                             start=False, stop=True)
            nc.vector.tensor_copy(zw_sb[:, half * D:(half + 1) * D], zw_ps[:])

        # ---- final: out_j = f_j * (x_j @ zw) ----
        out3 = out.rearrange("(p j) d -> p j d", p=P)
        o_all = pp.tile([P, TL * D], BF16)
        for i in range(TL):
            o_ps = psW.tile([P, D], F32, tag="pw", name=f"o{i}")
            for h in (1, 0):
                nc.tensor.matmul(
                    o_ps[:], lhsT=xbT[:, (2 * i + h) * P:(2 * i + h + 1) * P],
                    rhs=zw_sb[:, h * D:(h + 1) * D],
                    start=(h == 1), stop=(h == 0),
                )
            o_i = o_all[:, i * D:(i + 1) * D]
            if i % 2 == 0:
                nc.scalar.mul(o_i, o_ps[:], f_t[:, i:i + 1])
            else:
                nc.vector.tensor_scalar_mul(o_i, o_ps[:], f_t[:, i:i + 1])
            if i % 2 == 1:
                nc.sync.dma_start(
                    out3[:, i - 1:i + 1, :],
                    o_all[:, (i - 1) * D:(i + 1) * D].rearrange(
                        "p (j d) -> p j d", j=2),
                )


def _build():
    nc = bacc.Bacc("TRN2", target_bir_lowering=False, debug=False, num_devices=R)
    x = nc.dram_tensor("x", [N, D], BF16, kind="ExternalInput")
    xT = nc.dram_tensor("xT", [P, 2, N], FP8, kind="ExternalInput")
    W = nc.dram_tensor("W", [D, D], BF16, kind="ExternalInput")
    out = nc.dram_tensor("out", [NL, D], BF16, kind="ExternalOutput")
    with nc.allow_low_precision("bf16/fp8 feed; tol 2e-2"):
        with tile.TileContext(nc) as tc:
            _program(
                tc,
                x.ap() if hasattr(x, "ap") else x,
                xT.ap() if hasattr(xT, "ap") else xT,
                W.ap() if hasattr(W, "ap") else W,
                out.ap() if hasattr(out, "ap") else out,
            )
    nc.finalize()
    return nc


def _prep_host(x, W):
    xb = x.astype(ml_dtypes.bfloat16)
    Wb = W.astype(ml_dtypes.bfloat16)
    # row(p, i): chunk c=i//CH, j=i%CH -> row = c*1024 + 8p + j
    p_idx = np.arange(P)
    i_idx = np.arange(T)
    c = i_idx // CH
    j = i_idx % CH
    row_of_k = (c[None, :] * NL + 8 * p_idx[:, None] + j[None, :]).reshape(-1)
    return xb, Wb, row_of_k


def _run(inputs, trace=False):
    if "nc" not in _cache:
        _cache["nc"] = _build()
    nc = _cache["nc"]
    x = np.asarray(inputs["x"], dtype=np.float32)
    W = np.asarray(inputs["W"], dtype=np.float32)
    xb, Wb, row_of_k = _prep_host(x, W)
    in_maps = []
    for r in range(R):
        xr = np.roll(x, -r * NL, axis=0)
        xbr = np.ascontiguousarray(np.roll(xb, -r * NL, axis=0))
        x8 = xr[row_of_k, :].astype(ml_dtypes.float8_e4m3)     # [8192, 256]
        xT8 = np.ascontiguousarray(
            x8.T.reshape(2, P, N).transpose(1, 0, 2))          # [128, 2, 8192]
        in_maps.append({"x": xbr, "xT": xT8, "W": Wb})
    res = bass_utils.run_bass_kernel_spmd(
        nc, in_maps, core_ids=list(range(R)), trace=trace,
    )
    out = np.concatenate(
        [res.results[r]["out"].astype(np.float32) for r in range(R)], axis=0
    )
    return out, res


def kernel(**inputs) -> np.ndarray:
    out, _ = _run(inputs, trace=False)
    return out
